# revision 1
# baseline (speedup 1.0000x reference)
"""GAT recommender (2-layer GAT + residual + dot scoring) on 8 Trainium2 cores.

Strategy (edge-parallel, dst-sharded):
  - Sort edges by dst; core k owns a contiguous node range (W windows of 128
    nodes) and all edges whose dst falls in it.
  - Layer-1 gathers fetch 256B rows [x bf16 64 | e1_src 4xf32 | pad] by src
    (5 int16-indexed banks, dynamic counts rounded to 128); per-head
    attention-weighted x sums are scatter-accumulated into PSUM via one-hot
    PE matmuls; W1 is applied AFTER the scatter (linearity), then ELU and
    the layer-2 table row (x2 @ W2p, e2 projections) are produced in the
    same window tail -- no separate dense-1A/dense-2 phases over h1/x2.
  - exp(att - M) uses the upper bound M = lrelu(max e_src + max e_dst); the
    softmax normalization cancels the difference vs the reference's global
    max up to the 1e-8 epsilon (relative effect ~1e-9).
  - Layer 2 tables are AllGathered; final embeddings AllGathered; scoring
    dots computed on-device.
"""

import numpy as np

try:
    import concourse.bacc as bacc
except ImportError:  # harness fresh-dir fallback
    import sys
    for p in ("/opt/trn_rl_repo", "/root/.axon_site/_ro/trn_rl_repo"):
        if p not in sys.path:
            sys.path.insert(0, p)
    import concourse.bacc as bacc

import ml_dtypes
import concourse.mybir as mybir
import concourse.tile as tile
from concourse.bass import ds, IndirectOffsetOnAxis
from concourse.bass_isa import ReduceOp
from concourse.bass_utils import run_bass_kernel_spmd

P = 128
EMB = 64
H1 = 4          # layer-1 heads
NBANKS = 5
BANK = 30720    # int16-indexable bank rows (<= 32768)
CE1 = 128       # table1 row in bf16 units: [x 64 bf16 | e1_src 4 f32 | pad] -> 256B
CE1U = 260      # scatter columns (4 heads x 64 weighted-x + 4 att, bf16)
CE2 = 128       # table2 row in bf16 units: [h2 64 bf16 | e2_src,e2_dst f32] -> 256B
CE2U = 66       # fused dense-2 psum columns (h2 64 | e2_src | e2_dst, f32)
EPS = 1e-8
LRELU = 0.2
BF = ml_dtypes.bfloat16

F32 = mybir.dt.float32
BF16 = mybir.dt.bfloat16
I32 = mybir.dt.int32
I16 = mybir.dt.int16
AF = mybir.ActivationFunctionType
OP = mybir.AluOpType


def _leaky_max(nc, pool, ap, tag):
    """in-place x <- max(x, LRELU*x)"""
    shape = [ap.shape[0], int(np.prod(ap.shape[1:]))]
    tmp = pool.tile(shape, F32, tag=tag)
    nc.vector.tensor_scalar_mul(out=tmp[:], in0=ap, scalar1=LRELU)
    nc.vector.tensor_tensor(out=ap, in0=ap, in1=tmp[:], op=OP.max)


STATIC_COUNTS = "round128"


def build_program(cfg):
    """Builds the SPMD Bass program. cfg: dict with W, Tb, NSTAR, NC_NODES,
    NTAB, BB (batch per core), CORES, unrolls."""
    W, Tb, NB = cfg["W"], cfg["Tb"], cfg["NB"]
    NSTAR, NCN, NTAB = cfg["NSTAR"], cfg["NC_NODES"], cfg["NTAB"]
    BB, CORES = cfg["BB"], cfg["CORES"]
    TT = NB * Tb              # tiles per window
    NIDXB = Tb * P            # idxs per bank gather call
    ICOLS = NB * Tb * 8       # int16 idx columns per window
    DT1 = NSTAR // P          # dense-e tiles (all nodes)
    DTO = NCN // P            # dense tiles (own nodes)
    groups = [list(range(CORES))]

    nc = bacc.Bacc("TRN2", target_bir_lowering=False, debug=False,
                   num_devices=CORES)

    # ---------- inputs ----------
    xT_in = nc.dram_tensor("xT", [EMB, NSTAR], F32, kind="ExternalInput")
    xr_in = nc.dram_tensor("xr", [NSTAR, EMB], BF16, kind="ExternalInput")
    xTo_in = nc.dram_tensor("xTo", [EMB, NCN], F32, kind="ExternalInput")
    xo_in = nc.dram_tensor("xo", [NCN, EMB], F32, kind="ExternalInput")
    W1e_in = nc.dram_tensor("W1e", [EMB, 8], F32, kind="ExternalInput")
    W1b_in = nc.dram_tensor("W1b", [P, 2 * EMB], BF16, kind="ExternalInput")
    W2b_in = nc.dram_tensor("W2b", [4 * EMB, CE2U], BF16, kind="ExternalInput")
    iota_in = nc.dram_tensor("iota", [P, P], F32, kind="ExternalInput")
    ident_in = nc.dram_tensor("ident", [P, P], F32, kind="ExternalInput")
    dstw_in = nc.dram_tensor("dstw", [W * P, TT], F32, kind="ExternalInput")
    idxw_in = nc.dram_tensor("idxw", [W * P, ICOLS], I16, kind="ExternalInput")
    cnts_in = nc.dram_tensor("cnts", [W, NB], I32, kind="ExternalInput")
    cnts2_in = nc.dram_tensor("cnts2", [W, NB], I32, kind="ExternalInput")
    uidx_in = nc.dram_tensor("uidx", [P, BB // P], I32, kind="ExternalInput")
    iidx_in = nc.dram_tensor("iidx", [P, BB // P], I32, kind="ExternalInput")

    # ---------- outputs / intermediates ----------
    out_t = nc.dram_tensor("out", [P, BB // P], F32, kind="ExternalOutput")
    table1 = nc.dram_tensor("table1", [NTAB, CE1], BF16, kind="Internal")
    e1do = nc.dram_tensor("e1do", [NCN, H1], F32, kind="Internal")
    t2own = nc.dram_tensor("t2own", [NCN, CE2], BF16, kind="Internal")
    t2full = nc.dram_tensor("t2full", [NTAB, CE2], BF16, kind="Internal",
                            addr_space="Shared")
    m2loc = nc.dram_tensor("m2loc", [1, 2], F32, kind="Internal")
    m2sh = nc.dram_tensor("m2sh", [1, 2], F32, kind="Internal",
                          addr_space="Shared")
    hown = nc.dram_tensor("hown", [NCN, EMB], F32, kind="Internal")
    hfull = nc.dram_tensor("hfull", [CORES * NCN, EMB], F32, kind="Internal",
                           addr_space="Shared")

    with tile.TileContext(nc) as tc:
        with (
            tc.tile_pool(name="const", bufs=1) as cp,
            tc.tile_pool(name="sb", bufs=4) as sb,
            tc.tile_pool(name="gb", bufs=1) as gbp,
            tc.tile_pool(name="sall", bufs=4) as sap,
            tc.tile_pool(name="sg2", bufs=2) as sgp,
            tc.tile_pool(name="ps", bufs=2, space="PSUM") as pp,
            tc.tile_pool(name="pst", bufs=2, space="PSUM") as pst,
            tc.tile_pool(name="pw", bufs=2, space="PSUM") as pw,
            tc.tile_pool(name="ph", bufs=2, space="PSUM") as php,
        ):
            iota = cp.tile([P, P], F32)
            nc.sync.dma_start(out=iota[:], in_=iota_in[:, :])
            ident = cp.tile([P, P], F32)
            nc.sync.dma_start(out=ident[:], in_=ident_in[:, :])
            identb = cp.tile([P, P], BF16)
            nc.vector.tensor_copy(out=identb[:], in_=ident[:])
            W1e = cp.tile([EMB, 8], F32)
            nc.sync.dma_start(out=W1e[:], in_=W1e_in[:, :])
            # W1bs[p, blk, f] = W1[p % 64, (2*blk + p//64)*64 + f]: head h's
            # weights sit at partitions (h%2)*64.. so lhsT/rhs share base.
            W1bs = cp.tile([P, 2, EMB], BF16)
            nc.sync.dma_start(out=W1bs[:, :, :],
                              in_=W1b_in[:, :].rearrange("p (b f) -> p b f", b=2))
            W2bs = cp.tile([P, 2, CE2U], BF16)
            nc.sync.dma_start(out=W2bs[:, :, :],
                              in_=W2b_in[:, :].rearrange("(c p) n -> p c n", p=P))
            rmax1 = cp.tile([P, 8], F32)
            nc.vector.memset(rmax1[:], -3.0e38)
            rmax2 = cp.tile([P, 2], F32)
            nc.vector.memset(rmax2[:], -3.0e38)
            mneg1 = cp.tile([P, 1], F32)
            mneg2 = cp.tile([P, 1], F32)

            # ========== dense-e: table1 rows [x bf16 | e1_src] + e-max ======
            U = cfg["UN_D1"]
            with tc.For_i(0, DT1 // U) as i0:
                for k in range(U):
                    base = i0 * (U * P) + k * P
                    lt = sb.tile([EMB, P], F32, tag="d1l")
                    nc.sync.dma_start(out=lt[:], in_=xT_in[:, ds(base, P)])
                    xrt = sb.tile([P, EMB], BF16, tag="d1x")
                    nc.sync.dma_start(out=xrt[:], in_=xr_in[ds(base, P), :])
                    ep = pp.tile([P, 264], F32, tag="gp")
                    nc.tensor.matmul(out=ep[:, 0:8], lhsT=lt[:], rhs=W1e[:],
                                     start=True, stop=True)
                    row = sb.tile([P, CE1], BF16, tag="d1s")
                    nc.vector.tensor_copy(out=row[:, 0:EMB], in_=xrt[:])
                    nc.vector.tensor_copy(out=row[:, 64:72].bitcast(F32),
                                          in_=ep[:, 0:4])
                    nc.vector.memset(row[:, 72:128], 0.0)
                    nc.vector.tensor_tensor(out=rmax1[:], in0=rmax1[:],
                                            in1=ep[:, 0:8], op=OP.max)
                    nc.sync.dma_start(out=table1[ds(base, P), :],
                                      in_=row[:, :])

            # ========== dense-1B: own e1_dst ===============================
            U = cfg["UN_D1B"]
            with tc.For_i(0, DTO // U) as i1:
                for k in range(U):
                    lt = sb.tile([EMB, P], F32, tag="d1bl")
                    nc.sync.dma_start(out=lt[:],
                                      in_=xTo_in[:, ds(i1 * (U * P) + k * P, P)])
                    ep = pp.tile([P, 264], F32, tag="gp")
                    nc.tensor.matmul(out=ep[:, 0:H1], lhsT=lt[:],
                                     rhs=W1e[:, 4:8], start=True, stop=True)
                    es = sb.tile([P, H1], F32, tag="d1bs")
                    nc.vector.tensor_copy(out=es[:], in_=ep[:, 0:H1])
                    nc.sync.dma_start(out=e1do[ds(i1 * (U * P) + k * P, P), :],
                                      in_=es[:])

            # ========== M1 bound ===========================================
            rr1 = sb.tile([P, 8], F32, tag="rr1")
            nc.gpsimd.partition_all_reduce(rr1[:], rmax1[:], P, ReduceOp.max)
            ma = sb.tile([P, 1], F32, tag="ma")
            mb = sb.tile([P, 1], F32, tag="mb")
            nc.vector.reduce_max(ma[:], rr1[:, 0:4], axis=mybir.AxisListType.X)
            nc.vector.reduce_max(mb[:], rr1[:, 4:8], axis=mybir.AxisListType.X)
            nc.vector.tensor_tensor(out=ma[:], in0=ma[:], in1=mb[:], op=OP.add)
            _leaky_max(nc, sb, ma[:], "mlk")
            nc.vector.tensor_scalar_mul(out=mneg1[:], in0=ma[:], scalar1=-1.0)

            # zero the empty-group fallback rows that dense-e never writes
            # (only fires when a bank base lies past NSTAR; full-size configs
            # have none)
            zrow = sb.tile([P, CE1], BF16, tag="zrow")
            nc.vector.memset(zrow[:], 0.0)
            for b in range(NB):
                if b * BANK >= NSTAR:
                    nc.sync.dma_start(out=table1[b * BANK:b * BANK + 1, :],
                                      in_=zrow[0:1, :])
                    nc.sync.dma_start(out=t2full[b * BANK:b * BANK + 1, :],
                                      in_=zrow[0:1, 0:CE2])

            # gather-count registers (per unrolled window slot, per bank)
            UW = cfg["UN_WIN"]
            cregs = [[nc.gpsimd.alloc_register(f"cnt{k}_{b}")
                      for b in range(NB)] for k in range(UW)]

            # ========== GAT edge window ====================================
            def gat_window(w, lay, regs, gbuf, sgb):
                """One window of GAT message passing for layer `lay`.
                gbuf: gathered 256B rows; sgb (layer 1 only): scatter
                message buffer [P, TT, CE1U]."""
                NH = H1 if lay == 1 else 1
                XC = EMB                              # x/h cols in row
                CEu = XC * NH + NH                    # scatter cols
                tab = table1 if lay == 1 else t2full
                mneg = mneg1 if lay == 1 else mneg2
                tg = f"l{lay}"

                dstf = sb.tile([P, TT], F32, tag=tg + "dst")
                nc.sync.dma_start(out=dstf[:], in_=dstw_in[ds(w * P, P), :])
                idxt = sb.tile([P, ICOLS], I16, tag=tg + "idx")
                nc.sync.dma_start(out=idxt[:], in_=idxw_in[ds(w * P, P), :])
                cntt = sb.tile([1, NB], I32, tag=tg + "cnt")
                nc.sync.dma_start(
                    out=cntt[:],
                    in_=(cnts_in if lay == 1 else cnts2_in)[ds(w, 1), :])
                nc.gpsimd.reg_load(regs, cntt[0:1, 0:NB])
                edw = sb.tile([P, NH], F32, tag=tg + "edw")
                if lay == 1:
                    nc.sync.dma_start(out=edw[:], in_=e1do[ds(w * P, P), :])
                else:
                    nc.sync.dma_start(
                        out=edw[:],
                        in_=t2own[ds(w * P, P), 66:68].bitcast(F32))
                edwb = sb.tile([P, NH], BF16, tag=tg + "edwb")
                nc.vector.tensor_copy(out=edwb[:], in_=edw[:])

                for b in range(NB):
                    nc.gpsimd.dma_gather(
                        out_ap=gbuf[:, b * Tb:(b + 1) * Tb, :],
                        in_ap=tab[b * BANK:(b + 1) * BANK, :],
                        idxs_ap=idxt[:, b * Tb * 8:(b + 1) * Tb * 8],
                        num_idxs=NIDXB,
                        num_idxs_reg=regs[b],
                        elem_size=CE1 if lay == 1 else CE2,
                    )

                # one-hot S [slot, dst] per tile + transposed St for edp
                S_all = sap.tile([P, TT, P], BF16, tag=tg + "S")
                edp = pp.tile([P, 264], F32, tag="gp")
                for j in range(TT):
                    nc.vector.tensor_tensor(
                        out=S_all[:, j, :], in0=iota[:],
                        in1=dstf[:, j:j + 1].to_broadcast([P, P]),
                        op=OP.is_equal)
                    stp = pst.tile([P, P], BF16, tag="stp")
                    nc.tensor.transpose(out=stp[:], in_=S_all[:, j, :],
                                        identity=identb[:])
                    stb = sb.tile([P, P], BF16, tag=tg + "st")
                    nc.vector.tensor_copy(out=stb[:], in_=stp[:])
                    nc.tensor.matmul(out=edp[:, j * NH:(j + 1) * NH],
                                     lhsT=stb[:], rhs=edwb[:, :],
                                     start=True, stop=True)

                # att = exp(lrelu(e_src + e_dst) - M) -> message buffer
                mbuf = sgb if lay == 1 else gbuf
                acol = XC * NH if lay == 1 else XC
                att = sb.tile([P, TT * NH], F32, tag=tg + "att")
                nc.vector.tensor_tensor(
                    out=att[:].rearrange("p (t h) -> p t h", h=NH),
                    in0=gbuf[:, :, 64:64 + 2 * NH].bitcast(F32),
                    in1=edp[:, 0:TT * NH].rearrange("p (t h) -> p t h", h=NH),
                    op=OP.add)
                _leaky_max(nc, sb, att[:], tg + "alk")
                nc.scalar.activation(
                    mbuf[:, :, acol:acol + NH], att[:].rearrange(
                        "p (t h) -> p t h", h=NH),
                    AF.Exp, bias=mneg[:])
                if lay == 1:
                    nc.vector.tensor_tensor(
                        out=sgb[:, :, 0:XC * NH].rearrange(
                            "p t (h f) -> p t h f", h=NH),
                        in0=gbuf[:, :, 0:XC].rearrange(
                            "p t (o f) -> p t o f", o=1).to_broadcast(
                            [P, TT, NH, XC]),
                        in1=sgb[:, :, acol:acol + NH].rearrange(
                            "p t (h o) -> p t h o", o=1).to_broadcast(
                            [P, TT, NH, XC]),
                        op=OP.mult)
                else:
                    nc.vector.tensor_tensor(
                        out=gbuf[:, :, 0:XC],
                        in0=gbuf[:, :, 0:XC],
                        in1=gbuf[:, :, acol:acol + NH].to_broadcast(
                            [P, TT, XC]),
                        op=OP.mult)

                wps = pw.tile([P, CE1U], F32, tag="wp")
                for j in range(TT):
                    nc.tensor.matmul(out=wps[:, 0:CEu], lhsT=S_all[:, j, :],
                                     rhs=mbuf[:, j, 0:CEu],
                                     start=(j == 0), stop=(j == TT - 1))

                rec = sb.tile([P, NH], F32, tag=tg + "rec")
                nc.vector.tensor_scalar_add(out=rec[:],
                                            in0=wps[:, XC * NH:XC * NH + NH],
                                            scalar1=EPS)
                nc.vector.reciprocal(out=rec[:], in_=rec[:])
                return wps, rec

            U = cfg["UN_WIN"]

            def l1_tail(w, wps, rec):
                # normalized per-head weighted-x sums -> bf16
                onrb = sb.tile([P, H1 * EMB], BF16, tag="l1on")
                nc.vector.tensor_tensor(
                    out=onrb[:].rearrange("p (h f) -> p h f", h=H1),
                    in0=wps[:, 0:H1 * EMB].rearrange("p (h f) -> p h f", h=H1),
                    in1=rec[:].to_broadcast([P, H1, EMB]),
                    op=OP.mult)
                # transpose to [fin, dst] and apply W1 per head
                hps = php.tile([P, 2, P], F32, tag="hp")
                ont = sb.tile([P, 2, P], BF16, tag="l1ot")
                for blk in range(2):
                    otp = pst.tile([P, P], BF16, tag="stp")
                    nc.tensor.transpose(out=otp[:],
                                        in_=onrb[:, blk * P:(blk + 1) * P],
                                        identity=identb[:])
                    nc.vector.tensor_copy(out=ont[:, blk, :], in_=otp[:])
                for h in range(H1):
                    blk, half = h // 2, (h % 2) * EMB
                    nc.tensor.matmul(
                        out=hps[ds(half, EMB), blk, :],
                        lhsT=W1bs[ds(half, EMB), blk, :],
                        rhs=ont[ds(half, EMB), blk, :],
                        start=True, stop=True)
                # ELU -> x2T (bf16) ; then fused dense-2: h2 | e2 columns
                x2t = sb.tile([P, 2, P], BF16, tag="l1x2")
                for blk in range(2):
                    neg = sb.tile([P, P], F32, tag="l1ng")
                    nc.vector.tensor_scalar_min(out=neg[:],
                                                in0=hps[:, blk, :],
                                                scalar1=0.0)
                    nc.scalar.activation(neg[:], neg[:], AF.Exp)
                    pos = sb.tile([P, P], F32, tag="l1ps")
                    nc.vector.tensor_scalar_max(out=pos[:],
                                                in0=hps[:, blk, :],
                                                scalar1=0.0)
                    nc.vector.tensor_tensor(out=pos[:], in0=pos[:],
                                            in1=neg[:], op=OP.add)
                    nc.vector.tensor_scalar_add(out=x2t[:, blk, :],
                                                in0=pos[:], scalar1=-1.0)
                h2p = pp.tile([P, 264], F32, tag="gp")
                for c in range(2):
                    nc.tensor.matmul(out=h2p[:, 0:CE2U], lhsT=x2t[:, c, :],
                                     rhs=W2bs[:, c, :],
                                     start=(c == 0), stop=(c == 1))
                h2s = sb.tile([P, 68], BF16, tag="l1h2")
                nc.vector.tensor_copy(out=h2s[:, 0:64], in_=h2p[:, 0:64])
                nc.vector.tensor_copy(out=h2s[:, 64:68].bitcast(F32),
                                      in_=h2p[:, 64:66])
                nc.vector.tensor_tensor(out=rmax2[:], in0=rmax2[:],
                                        in1=h2p[:, 64:66], op=OP.max)
                nc.sync.dma_start(out=t2own[ds(w * P, P), 0:68],
                                  in_=h2s[:, 0:68])

            NGB = cfg.get("NGBUF", 2)
            g1bufs = [gbp.tile([P, TT, CE1], BF16, tag=f"g1_{i}", name=f"g1_{i}")
                      for i in range(NGB)]
            for g in g1bufs:
                nc.vector.memset(g[:, :, :], 0.0)

            def win1(w, k):
                sgb = sgp.tile([P, TT, CE1U], BF16, tag="sgb")
                wps, rec = gat_window(w, 1, cregs[k], g1bufs[k % NGB], sgb)
                l1_tail(w, wps, rec)

            for k in range(U):  # peeled (program warms the buffers)
                win1(k, k)
            with tc.For_i(1, W // U) as w0:
                for k in range(U):
                    win1(w0 * U + k, k)

            # ========== M2 bound (allreduce) + table2 allgather ============
            rr2 = sb.tile([P, 2], F32, tag="rr2")
            nc.gpsimd.partition_all_reduce(rr2[:], rmax2[:], P, ReduceOp.max)
            nc.sync.dma_start(out=m2loc[:, :], in_=rr2[0:1, :])
            nc.gpsimd.collective_compute(
                "AllReduce", OP.max, replica_groups=groups,
                ins=[m2loc[:, :]], outs=[m2sh[:, :]])
            m2t = sb.tile([P, 2], F32, tag="m2t")
            nc.sync.dma_start(out=m2t[:], in_=m2sh[:, :].to_broadcast([P, 2]))
            nc.vector.tensor_tensor(out=m2t[:, 0:1], in0=m2t[:, 0:1],
                                    in1=m2t[:, 1:2], op=OP.add)
            _leaky_max(nc, sb, m2t[:, 0:1], "m2lk")
            nc.vector.tensor_scalar_mul(out=mneg2[:], in0=m2t[:, 0:1],
                                        scalar1=-1.0)

            nc.gpsimd.collective_compute(
                "AllGather", OP.bypass, replica_groups=groups,
                ins=[t2own[:, :]], outs=[t2full[0:CORES * NCN, :]])

            # ========== layer-2 edge windows ===============================
            def l2_tail(w, wps, rec):
                onr = sb.tile([P, EMB], F32, tag="l2on")
                nc.vector.tensor_tensor(
                    out=onr[:], in0=wps[:, 0:EMB],
                    in1=rec[:].to_broadcast([P, EMB]), op=OP.mult)
                xot = sb.tile([P, EMB], F32, tag="l2xo")
                nc.sync.dma_start(out=xot[:], in_=xo_in[ds(w * P, P), :])
                nc.vector.tensor_tensor(out=xot[:], in0=xot[:], in1=onr[:],
                                        op=OP.add)
                nc.sync.dma_start(out=hown[ds(w * P, P), :], in_=xot[:])

            g2bufs = [gbp.tile([P, TT, CE2], BF16, tag=f"g2_{i}", name=f"g2_{i}")
                      for i in range(NGB)]
            for g in g2bufs:
                nc.vector.memset(g[:, :, :], 0.0)

            def win2(w, k):
                wps, rec = gat_window(w, 2, cregs[k], g2bufs[k % NGB], None)
                l2_tail(w, wps, rec)

            for k in range(U):
                win2(k, k)
            with tc.For_i(1, W // U) as w1:
                for k in range(U):
                    win2(w1 * U + k, k)

            # ========== final embeddings allgather + dots ==================
            nc.gpsimd.collective_compute(
                "AllGather", OP.bypass, replica_groups=groups,
                ins=[hown[:, :]], outs=[hfull[:, :]])

            uix = sb.tile([P, BB // P], I32, tag="uix")
            nc.sync.dma_start(out=uix[:], in_=uidx_in[:, :])
            iix = sb.tile([P, BB // P], I32, tag="iix")
            nc.sync.dma_start(out=iix[:], in_=iidx_in[:, :])
            ubuf = gbp.tile([P, BB // P, EMB], F32, tag="ubuf", name="ubuf")
            ibuf = gbp.tile([P, BB // P, EMB], F32, tag="ibuf", name="ibuf")
            for j in range(BB // P):
                nc.gpsimd.indirect_dma_start(
                    out=ubuf[:, j, :], out_offset=None, in_=hfull[:, :],
                    in_offset=IndirectOffsetOnAxis(ap=uix[:, j:j + 1], axis=0))
                nc.gpsimd.indirect_dma_start(
                    out=ibuf[:, j, :], out_offset=None, in_=hfull[:, :],
                    in_offset=IndirectOffsetOnAxis(ap=iix[:, j:j + 1], axis=0))
            nc.vector.tensor_tensor(
                out=ubuf[:, :, :], in0=ubuf[:, :, :], in1=ibuf[:, :, :],
                op=OP.mult)
            dots = sb.tile([P, BB // P], F32, tag="dots")
            nc.vector.reduce_sum(dots[:], ubuf[:, :, :],
                                 axis=mybir.AxisListType.X)
            nc.sync.dma_start(out=out_t[:, :], in_=dots[:])

    nc.compile()
    return nc


def prepare_inputs(user_table, item_table, W1, a1, W2, a2, edge_index,
                   user_ids, item_ids, cfg):
    W, CORES = cfg["W"], cfg["CORES"]
    NSTAR, NCN, BB = cfg["NSTAR"], cfg["NC_NODES"], cfg["BB"]
    NU = user_table.shape[0]
    N = NU + item_table.shape[0]

    x = np.concatenate([np.asarray(user_table, np.float32),
                        np.asarray(item_table, np.float32)], axis=0)
    xpad = np.zeros((NSTAR, EMB), np.float32)
    xpad[:N] = x
    xT = np.ascontiguousarray(xpad.T)
    xr = xpad.astype(BF)

    W1 = np.asarray(W1, np.float32)
    a1 = np.asarray(a1, np.float32)
    W2 = np.asarray(W2, np.float32)
    a2 = np.asarray(a2, np.float32)
    A1l = np.stack([W1[:, h * EMB:(h + 1) * EMB] @ a1[h, :EMB]
                    for h in range(H1)], axis=1)
    A1r = np.stack([W1[:, h * EMB:(h + 1) * EMB] @ a1[h, EMB:]
                    for h in range(H1)], axis=1)
    W1e = np.concatenate([A1l, A1r], axis=1)          # [64, 8]
    # W1b[p, blk*64+f] = W1[p%64, (2*blk + p//64)*64 + f]  (head h at
    # partitions (h%2)*64 so the per-head matmul lhsT/rhs bases match)
    W1b = np.zeros((P, 2 * EMB), np.float32)
    for h in range(H1):
        blk, half = h // 2, (h % 2) * EMB
        W1b[half:half + EMB, blk * EMB:(blk + 1) * EMB] = \
            W1[:, h * EMB:(h + 1) * EMB]
    W1b = W1b.astype(BF)
    w2l = W2 @ a2[0, :EMB]
    w2r = W2 @ a2[0, EMB:]
    W2b = np.concatenate([W2, w2l[:, None], w2r[:, None]], axis=1).astype(BF)

    src = np.asarray(edge_index[0]).astype(np.int64)
    dst = np.asarray(edge_index[1]).astype(np.int64)
    NB = cfg["NB"]
    NWG = CORES * W
    key = (dst // P) * NB + (src // BANK)
    order = np.argsort(key, kind="stable")
    src_g, dst_g, key_g = src[order], dst[order], key[order]
    cnt = np.bincount(key_g, minlength=NWG * NB)
    Tb = cfg["Tb"]
    assert cnt.max() <= Tb * P, f"Tb={Tb} too small for {cnt.max()}"
    NIDXB = Tb * P
    gstart = np.concatenate([[0], np.cumsum(cnt)])[:-1]
    off = np.arange(len(src_g)) - gstart[key_g]
    slot = key_g * NIDXB + off
    bankidx = (src_g - (src_g // BANK) * BANK).astype(np.int16)

    def idx_layout(flat):
        A = flat.reshape(NWG, NB, Tb * 8, 16)
        A = np.transpose(A, (0, 1, 3, 2))                  # [wg, b, 16, cols]
        A = np.tile(A, (1, 1, 8, 1))                       # [wg, b, 128, cols]
        return np.ascontiguousarray(
            np.transpose(A, (0, 2, 1, 3)).reshape(NWG, P, NB * Tb * 8))

    if STATIC_COUNTS == "round128":
        # Dynamic counts rounded up to a multiple of 128: every SDMA engine
        # participates (sem reaches 16). Contract (bass_interp:3897): the
        # count register == #non-negative idxs and all idxs past it are -1.
        # Positions [cnt, cnt_r) gather row 0 (valid); tiles past cnt_r stay
        # stale-but-finite and their dstf=-1 zeroes their S columns.
        cnts = np.clip(cnt.reshape(NWG, NB), 128, NIDXB).astype(np.int32)
        idx_flatn = np.full(NWG * NB * NIDXB, -1, np.int16)
        idx_flatn[slot] = bankidx
        pos = np.tile(np.arange(NIDXB, dtype=np.int32), NWG * NB)
        inpad = (pos < cnts.reshape(-1).repeat(NIDXB)) & (idx_flatn < 0)
        idx_flatn[inpad] = 0
    else:
        idx_flatn = np.zeros(NWG * NB * NIDXB, np.int16)
        idx_flatn[slot] = bankidx
        cnts = np.full((NWG, NB), NIDXB, np.int32)
    idx_dma_n = idx_layout(idx_flatn)
    dst_flat = np.full(NWG * NB * NIDXB, -1.0, np.float32)
    dst_flat[slot] = (dst_g % P).astype(np.float32)
    dst_dma = np.ascontiguousarray(
        dst_flat.reshape(NWG, NB * Tb, P).transpose(0, 2, 1))

    iota_np = np.tile(np.arange(P, dtype=np.float32), (P, 1))
    ident_np = np.eye(P, dtype=np.float32)

    uids = np.asarray(user_ids).astype(np.int64)
    iids = np.asarray(item_ids).astype(np.int64) + NU

    in_maps = []
    for k in range(CORES):
        in_maps.append(dict(
            xT=xT,
            xr=xr,
            xTo=np.ascontiguousarray(xT[:, k * NCN:(k + 1) * NCN]),
            xo=np.ascontiguousarray(xpad[k * NCN:(k + 1) * NCN]),
            W1e=W1e, W1b=W1b, W2b=W2b, iota=iota_np, ident=ident_np,
            dstw=idx_dst(idx_dma_n, dst_dma, k, W)[1],
            idxw=idx_dst(idx_dma_n, dst_dma, k, W)[0],
            cnts=np.ascontiguousarray(cnts[k * W:(k + 1) * W]),
            cnts2=np.ascontiguousarray(cnts[k * W:(k + 1) * W]),
            uidx=uids[k * cfg["BB"]:(k + 1) * cfg["BB"]].astype(
                np.int32).reshape(P, BB // P),
            iidx=iids[k * cfg["BB"]:(k + 1) * cfg["BB"]].astype(
                np.int32).reshape(P, BB // P),
        ))
    return in_maps


def idx_dst(idx_dma, dst_dma, k, W):
    i = np.ascontiguousarray(
        idx_dma[k * W:(k + 1) * W].reshape(W * P, -1))
    d = np.ascontiguousarray(
        dst_dma[k * W:(k + 1) * W].reshape(W * P, -1))
    return i, d


DEFAULT_CFG = dict(
    CORES=8, W=148, NC_NODES=148 * P, NSTAR=8 * 148 * P, NTAB=NBANKS * BANK,
    NB=5, Tb=4, BB=2048, UN_D1=16, UN_D1B=4, UN_WIN=4, UN_D2=4, NGBUF=4,
)

_PROGRAM_CACHE = {}


def _get_program(cfg_key, cfg):
    if cfg_key not in _PROGRAM_CACHE:
        _PROGRAM_CACHE[cfg_key] = build_program(cfg)
    return _PROGRAM_CACHE[cfg_key]


def run(inputs, cfg=None, trace=False):
    cfg = dict(DEFAULT_CFG if cfg is None else cfg)
    # size Tb from the data (static program structure depends on it)
    src = np.asarray(inputs["edge_index"][0]).astype(np.int64)
    dst = np.asarray(inputs["edge_index"][1]).astype(np.int64)
    key = (dst // P) * cfg["NB"] + (src // BANK)
    cnt = np.bincount(key, minlength=cfg["CORES"] * cfg["W"] * cfg["NB"])
    cfg["Tb"] = max(int(np.ceil(cnt.max() / P)), 1)
    in_maps = prepare_inputs(cfg=cfg, **inputs)
    nc = _get_program(tuple(sorted(cfg.items())), cfg)
    res = run_bass_kernel_spmd(nc, in_maps,
                               core_ids=list(range(cfg["CORES"])),
                               trace=trace)
    outs = [res.results[k]["out"].reshape(-1) for k in range(cfg["CORES"])]
    return np.concatenate(outs).astype(np.float32), res


def kernel(user_table, item_table, W1, a1, W2, a2, edge_index, user_ids,
           item_ids):
    out, _ = run(dict(user_table=user_table, item_table=item_table, W1=W1,
                      a1=a1, W2=W2, a2=a2, edge_index=edge_index,
                      user_ids=user_ids, item_ids=item_ids))
    return out



# revision 12
# speedup vs baseline: 1.0439x; 1.0439x over previous
"""GAT recommender (2-layer GAT + residual + dot scoring) on 8 Trainium2 cores.

Strategy (edge-parallel, dst-sharded):
  - Sort edges by dst; core k owns a contiguous node range (W windows of 128
    nodes) and all edges whose dst falls in it.
  - Layer-1 gathers fetch 256B rows [x bf16 64 | e1_src 4xf32 | pad] by src
    (5 int16-indexed banks, dynamic counts rounded to 128); per-head
    attention-weighted x sums are scatter-accumulated into PSUM via one-hot
    PE matmuls; W1 is applied AFTER the scatter (linearity), then ELU and
    the layer-2 table row (x2 @ W2p, e2 projections) are produced in the
    same window tail -- no separate dense-1A/dense-2 phases over h1/x2.
  - exp(att - M) uses the upper bound M = lrelu(max e_src + max e_dst); the
    softmax normalization cancels the difference vs the reference's global
    max up to the 1e-8 epsilon (relative effect ~1e-9).
  - Layer 2 tables are AllGathered; final embeddings AllGathered; scoring
    dots computed on-device.
"""

import numpy as np

try:
    import concourse.bacc as bacc
except ImportError:  # harness fresh-dir fallback
    import sys
    for p in ("/opt/trn_rl_repo", "/root/.axon_site/_ro/trn_rl_repo"):
        if p not in sys.path:
            sys.path.insert(0, p)
    import concourse.bacc as bacc

import ml_dtypes
import concourse.mybir as mybir
import concourse.tile as tile
from concourse.bass import ds, IndirectOffsetOnAxis
from concourse.bass_isa import ReduceOp
from concourse.bass_utils import run_bass_kernel_spmd

P = 128
EMB = 64
H1 = 4          # layer-1 heads
NBANKS = 5
BANK = 30720    # int16-indexable bank rows (<= 32768)
CE1 = 128       # table1 row in bf16 units: [x 64 bf16 | e1_src 4 f32 | pad] -> 256B
CE1U = 260      # scatter columns (4 heads x 64 weighted-x + 4 att, bf16)
CE2 = 128       # table2 row in bf16 units: [h2 64 bf16 | e2_src,e2_dst f32] -> 256B
CE2U = 66       # fused dense-2 psum columns (h2 64 | e2_src | e2_dst, f32)
EPS = 1e-8
LRELU = 0.2
BF = ml_dtypes.bfloat16

F32 = mybir.dt.float32
BF16 = mybir.dt.bfloat16
I32 = mybir.dt.int32
I16 = mybir.dt.int16
AF = mybir.ActivationFunctionType
OP = mybir.AluOpType


def _leaky_max(nc, pool, ap, tag):
    """in-place x <- max(x, LRELU*x)"""
    shape = [ap.shape[0], int(np.prod(ap.shape[1:]))]
    tmp = pool.tile(shape, F32, tag=tag)
    nc.vector.tensor_scalar_mul(out=tmp[:], in0=ap, scalar1=LRELU)
    nc.vector.tensor_tensor(out=ap, in0=ap, in1=tmp[:], op=OP.max)


STATIC_COUNTS = "round128"


def build_program(cfg):
    """Builds the SPMD Bass program. cfg: dict with W, Tb, NSTAR, NC_NODES,
    NTAB, BB (batch per core), CORES, unrolls."""
    W, Tb, NB = cfg["W"], cfg["Tb"], cfg["NB"]
    NSTAR, NCN, NTAB = cfg["NSTAR"], cfg["NC_NODES"], cfg["NTAB"]
    BB, CORES = cfg["BB"], cfg["CORES"]
    ABL = cfg.get("ABLATE", "full")
    TT = NB * Tb              # tiles per window
    NIDXB = Tb * P            # idxs per bank gather call
    ICOLS = NB * Tb * 8       # int16 idx columns per window
    DT1 = NSTAR // P          # dense-e tiles (all nodes)
    DTO = NCN // P            # dense tiles (own nodes)
    groups = [list(range(CORES))]

    NQ = cfg.get("NQ", 1)
    nc = bacc.Bacc("TRN2", target_bir_lowering=False, debug=False,
                   num_devices=CORES, num_swdge_queues=NQ)

    # ---------- inputs ----------
    xT_in = nc.dram_tensor("xT", [EMB, NSTAR], F32, kind="ExternalInput")
    xr_in = nc.dram_tensor("xr", [NSTAR, EMB], BF16, kind="ExternalInput")
    xTo_in = nc.dram_tensor("xTo", [EMB, NCN], F32, kind="ExternalInput")
    xo_in = nc.dram_tensor("xo", [NCN, EMB], F32, kind="ExternalInput")
    W1e_in = nc.dram_tensor("W1e", [EMB, 8], F32, kind="ExternalInput")
    W1b_in = nc.dram_tensor("W1b", [P, 2 * EMB], BF16, kind="ExternalInput")
    W2b_in = nc.dram_tensor("W2b", [4 * EMB, CE2U], BF16, kind="ExternalInput")
    iota_in = nc.dram_tensor("iota", [P, P], F32, kind="ExternalInput")
    ident_in = nc.dram_tensor("ident", [P, P], F32, kind="ExternalInput")
    dstw_in = nc.dram_tensor("dstw", [W * P, TT], F32, kind="ExternalInput")
    idxw_in = nc.dram_tensor("idxw", [W * P, ICOLS], I16, kind="ExternalInput")
    cnts_in = nc.dram_tensor("cnts", [W, NB], I32, kind="ExternalInput")
    cnts2_in = nc.dram_tensor("cnts2", [W, NB], I32, kind="ExternalInput")
    uidx_in = nc.dram_tensor("uidx", [P, BB // P], I32, kind="ExternalInput")
    iidx_in = nc.dram_tensor("iidx", [P, BB // P], I32, kind="ExternalInput")

    # ---------- outputs / intermediates ----------
    out_t = nc.dram_tensor("out", [P, BB // P], F32, kind="ExternalOutput")
    table1 = nc.dram_tensor("table1", [NTAB, CE1], BF16, kind="Internal")
    e1do = nc.dram_tensor("e1do", [NCN, H1], F32, kind="Internal")
    t2own = nc.dram_tensor("t2own", [NCN, CE2], BF16, kind="Internal")
    t2full = nc.dram_tensor("t2full", [NTAB, CE2], BF16, kind="Internal",
                            addr_space="Shared")
    m2loc = nc.dram_tensor("m2loc", [1, 2], F32, kind="Internal")
    m2sh = nc.dram_tensor("m2sh", [1, 2], F32, kind="Internal",
                          addr_space="Shared")
    hown = nc.dram_tensor("hown", [NCN, EMB], F32, kind="Internal")
    hfull = nc.dram_tensor("hfull", [CORES * NCN, EMB], F32, kind="Internal",
                           addr_space="Shared")

    with tile.TileContext(nc) as tc:
        with (
            tc.tile_pool(name="const", bufs=1) as cp,
            tc.tile_pool(name="sb", bufs=4) as sb,
            tc.tile_pool(name="gb", bufs=1) as gbp,
            tc.tile_pool(name="sall", bufs=4) as sap,
            tc.tile_pool(name="sg2", bufs=2) as sgp,
            tc.tile_pool(name="ps", bufs=2, space="PSUM") as pp,
            tc.tile_pool(name="pst", bufs=2, space="PSUM") as pst,
            tc.tile_pool(name="pw", bufs=2, space="PSUM") as pw,
            tc.tile_pool(name="ph", bufs=2, space="PSUM") as php,
        ):
            iota = cp.tile([P, P], F32)
            nc.sync.dma_start(out=iota[:], in_=iota_in[:, :])
            ident = cp.tile([P, P], F32)
            nc.sync.dma_start(out=ident[:], in_=ident_in[:, :])
            identb = cp.tile([P, P], BF16)
            nc.vector.tensor_copy(out=identb[:], in_=ident[:])
            W1e = cp.tile([EMB, 8], F32)
            nc.sync.dma_start(out=W1e[:], in_=W1e_in[:, :])
            # W1bs[p, blk, f] = W1[p % 64, (2*blk + p//64)*64 + f]: head h's
            # weights sit at partitions (h%2)*64.. so lhsT/rhs share base.
            W1bs = cp.tile([P, 2, EMB], BF16)
            nc.sync.dma_start(out=W1bs[:, :, :],
                              in_=W1b_in[:, :].rearrange("p (b f) -> p b f", b=2))
            W2bs = cp.tile([P, 2, CE2U], BF16)
            nc.sync.dma_start(out=W2bs[:, :, :],
                              in_=W2b_in[:, :].rearrange("(c p) n -> p c n", p=P))
            rmax1 = cp.tile([P, 8], F32)
            nc.vector.memset(rmax1[:], -3.0e38)
            rmax2 = cp.tile([P, 2], F32)
            nc.vector.memset(rmax2[:], -3.0e38)
            mneg1 = cp.tile([P, 1], F32)
            mneg2 = cp.tile([P, 1], F32)

            # ========== dense-e: table1 rows [x bf16 | e1_src] + e-max ======
            U = cfg["UN_D1"]
            with tc.For_i(0, DT1 // U) as i0:
                for k in range(U):
                    base = i0 * (U * P) + k * P
                    lt = sb.tile([EMB, P], F32, tag="d1l")
                    nc.sync.dma_start(out=lt[:], in_=xT_in[:, ds(base, P)])
                    xrt = sb.tile([P, EMB], BF16, tag="d1x")
                    nc.sync.dma_start(out=xrt[:], in_=xr_in[ds(base, P), :])
                    ep = pp.tile([P, 264], F32, tag="gp")
                    nc.tensor.matmul(out=ep[:, 0:8], lhsT=lt[:], rhs=W1e[:],
                                     start=True, stop=True)
                    row = sb.tile([P, CE1], BF16, tag="d1s")
                    nc.vector.tensor_copy(out=row[:, 0:EMB], in_=xrt[:])
                    nc.vector.tensor_copy(out=row[:, 64:72].bitcast(F32),
                                          in_=ep[:, 0:4])
                    nc.vector.memset(row[:, 72:128], 0.0)
                    nc.vector.tensor_tensor(out=rmax1[:], in0=rmax1[:],
                                            in1=ep[:, 0:8], op=OP.max)
                    nc.sync.dma_start(out=table1[ds(base, P), :],
                                      in_=row[:, :])

            # ========== dense-1B: own e1_dst ===============================
            U = cfg["UN_D1B"]
            with tc.For_i(0, DTO // U) as i1:
                for k in range(U):
                    lt = sb.tile([EMB, P], F32, tag="d1bl")
                    nc.sync.dma_start(out=lt[:],
                                      in_=xTo_in[:, ds(i1 * (U * P) + k * P, P)])
                    ep = pp.tile([P, 264], F32, tag="gp")
                    nc.tensor.matmul(out=ep[:, 0:H1], lhsT=lt[:],
                                     rhs=W1e[:, 4:8], start=True, stop=True)
                    es = sb.tile([P, H1], F32, tag="d1bs")
                    nc.vector.tensor_copy(out=es[:], in_=ep[:, 0:H1])
                    nc.sync.dma_start(out=e1do[ds(i1 * (U * P) + k * P, P), :],
                                      in_=es[:])

            # ========== M1 bound ===========================================
            rr1 = sb.tile([P, 8], F32, tag="rr1")
            nc.gpsimd.partition_all_reduce(rr1[:], rmax1[:], P, ReduceOp.max)
            ma = sb.tile([P, 1], F32, tag="ma")
            mb = sb.tile([P, 1], F32, tag="mb")
            nc.vector.reduce_max(ma[:], rr1[:, 0:4], axis=mybir.AxisListType.X)
            nc.vector.reduce_max(mb[:], rr1[:, 4:8], axis=mybir.AxisListType.X)
            nc.vector.tensor_tensor(out=ma[:], in0=ma[:], in1=mb[:], op=OP.add)
            _leaky_max(nc, sb, ma[:], "mlk")
            nc.vector.tensor_scalar_mul(out=mneg1[:], in0=ma[:], scalar1=-1.0)

            # zero the empty-group fallback rows that dense-e never writes
            # (only fires when a bank base lies past NSTAR; full-size configs
            # have none)
            zrow = sb.tile([P, CE1], BF16, tag="zrow")
            nc.vector.memset(zrow[:], 0.0)
            for b in range(NB):
                if b * BANK >= NSTAR:
                    nc.sync.dma_start(out=table1[b * BANK:b * BANK + 1, :],
                                      in_=zrow[0:1, :])
                    nc.sync.dma_start(out=t2full[b * BANK:b * BANK + 1, :],
                                      in_=zrow[0:1, 0:CE2])

            # gather-count registers (per unrolled window slot, per bank)
            UW = cfg["UN_WIN"]
            cregs = [[nc.gpsimd.alloc_register(f"cnt{k}_{b}")
                      for b in range(NB)] for k in range(UW)]

            # ========== GAT edge window ====================================
            def gat_window(w, lay, regs, gbuf, sgb, qbase=0):
                """One window of GAT message passing for layer `lay`.
                gbuf: gathered 256B rows; sgb (layer 1 only): scatter
                message buffer [P, TT, CE1U]."""
                NH = H1 if lay == 1 else 1
                XC = EMB                              # x/h cols in row
                CEu = XC * NH + NH                    # scatter cols
                tab = table1 if lay == 1 else t2full
                mneg = mneg1 if lay == 1 else mneg2
                tg = f"l{lay}"

                dstf = sb.tile([P, TT], F32, tag=tg + "dst")
                nc.sync.dma_start(out=dstf[:], in_=dstw_in[ds(w * P, P), :])
                idxt = sb.tile([P, ICOLS], I16, tag=tg + "idx")
                nc.sync.dma_start(out=idxt[:], in_=idxw_in[ds(w * P, P), :])
                cntt = sb.tile([1, NB], I32, tag=tg + "cnt")
                nc.sync.dma_start(
                    out=cntt[:],
                    in_=(cnts_in if lay == 1 else cnts2_in)[ds(w, 1), :])
                nc.gpsimd.reg_load(regs, cntt[0:1, 0:NB])
                edw = sb.tile([P, NH], F32, tag=tg + "edw")
                if lay == 1:
                    nc.sync.dma_start(out=edw[:], in_=e1do[ds(w * P, P), :])
                else:
                    nc.sync.dma_start(
                        out=edw[:],
                        in_=t2own[ds(w * P, P), 66:68].bitcast(F32))
                edwb = sb.tile([P, NH], BF16, tag=tg + "edwb")
                nc.vector.tensor_copy(out=edwb[:], in_=edw[:])

                if ABL != "nogather":
                    for b in range(NB):
                        nc.gpsimd.dma_gather(
                            out_ap=gbuf[:, b * Tb:(b + 1) * Tb, :],
                            in_ap=tab[b * BANK:(b + 1) * BANK, :],
                            idxs_ap=idxt[:, b * Tb * 8:(b + 1) * Tb * 8],
                            num_idxs=NIDXB,
                            num_idxs_reg=regs[b],
                            elem_size=CE1 if lay == 1 else CE2,
                            queue_num=(qbase * NB + b) % NQ,
                        )
                if ABL == "nowinc":
                    return None, None

                # one-hot S [slot, dst] per tile + transposed St for edp
                S_all = sap.tile([P, TT, P], BF16, tag=tg + "S")
                edp = pp.tile([P, 264], F32, tag="gp")
                for j in range(TT):
                    nc.vector.tensor_tensor(
                        out=S_all[:, j, :], in0=iota[:],
                        in1=dstf[:, j:j + 1].to_broadcast([P, P]),
                        op=OP.is_equal)
                    stp = pst.tile([P, P], BF16, tag="stp")
                    nc.tensor.transpose(out=stp[:], in_=S_all[:, j, :],
                                        identity=identb[:])
                    stb = sb.tile([P, P], BF16, tag=tg + "st")
                    nc.vector.tensor_copy(out=stb[:], in_=stp[:])
                    nc.tensor.matmul(out=edp[:, j * NH:(j + 1) * NH],
                                     lhsT=stb[:], rhs=edwb[:, :],
                                     start=True, stop=True)

                # att = exp(lrelu(e_src + e_dst) - M) -> message buffer
                mbuf = sgb if lay == 1 else gbuf
                acol = XC * NH if lay == 1 else XC
                att = sb.tile([P, TT * NH], F32, tag=tg + "att")
                nc.vector.tensor_tensor(
                    out=att[:].rearrange("p (t h) -> p t h", h=NH),
                    in0=gbuf[:, :, 64:64 + 2 * NH].bitcast(F32),
                    in1=edp[:, 0:TT * NH].rearrange("p (t h) -> p t h", h=NH),
                    op=OP.add)
                _leaky_max(nc, sb, att[:], tg + "alk")
                nc.scalar.activation(
                    mbuf[:, :, acol:acol + NH], att[:].rearrange(
                        "p (t h) -> p t h", h=NH),
                    AF.Exp, bias=mneg[:])
                if lay == 1:
                    nc.vector.tensor_tensor(
                        out=sgb[:, :, 0:XC * NH].rearrange(
                            "p t (h f) -> p t h f", h=NH),
                        in0=gbuf[:, :, 0:XC].rearrange(
                            "p t (o f) -> p t o f", o=1).to_broadcast(
                            [P, TT, NH, XC]),
                        in1=sgb[:, :, acol:acol + NH].rearrange(
                            "p t (h o) -> p t h o", o=1).to_broadcast(
                            [P, TT, NH, XC]),
                        op=OP.mult)
                else:
                    nc.vector.tensor_tensor(
                        out=gbuf[:, :, 0:XC],
                        in0=gbuf[:, :, 0:XC],
                        in1=gbuf[:, :, acol:acol + NH].to_broadcast(
                            [P, TT, XC]),
                        op=OP.mult)

                wps = pw.tile([P, CE1U], F32, tag="wp")
                for j in range(TT):
                    nc.tensor.matmul(out=wps[:, 0:CEu], lhsT=S_all[:, j, :],
                                     rhs=mbuf[:, j, 0:CEu],
                                     start=(j == 0), stop=(j == TT - 1))

                rec = sb.tile([P, NH], F32, tag=tg + "rec")
                nc.vector.tensor_scalar_add(out=rec[:],
                                            in0=wps[:, XC * NH:XC * NH + NH],
                                            scalar1=EPS)
                nc.vector.reciprocal(out=rec[:], in_=rec[:])
                return wps, rec

            U = cfg["UN_WIN"]

            def l1_tail(w, wps, rec):
                # normalized per-head weighted-x sums -> bf16
                onrb = sb.tile([P, H1 * EMB], BF16, tag="l1on")
                nc.vector.tensor_tensor(
                    out=onrb[:].rearrange("p (h f) -> p h f", h=H1),
                    in0=wps[:, 0:H1 * EMB].rearrange("p (h f) -> p h f", h=H1),
                    in1=rec[:].to_broadcast([P, H1, EMB]),
                    op=OP.mult)
                # transpose to [fin, dst] and apply W1 per head
                hps = php.tile([P, 2, P], F32, tag="hp")
                ont = sb.tile([P, 2, P], BF16, tag="l1ot")
                for blk in range(2):
                    otp = pst.tile([P, P], BF16, tag="stp")
                    nc.tensor.transpose(out=otp[:],
                                        in_=onrb[:, blk * P:(blk + 1) * P],
                                        identity=identb[:])
                    nc.vector.tensor_copy(out=ont[:, blk, :], in_=otp[:])
                for h in range(H1):
                    blk, half = h // 2, (h % 2) * EMB
                    nc.tensor.matmul(
                        out=hps[ds(half, EMB), blk, :],
                        lhsT=W1bs[ds(half, EMB), blk, :],
                        rhs=ont[ds(half, EMB), blk, :],
                        start=True, stop=True)
                # ELU -> x2T (bf16) ; then fused dense-2: h2 | e2 columns
                x2t = sb.tile([P, 2, P], BF16, tag="l1x2")
                for blk in range(2):
                    neg = sb.tile([P, P], F32, tag="l1ng")
                    nc.vector.tensor_scalar_min(out=neg[:],
                                                in0=hps[:, blk, :],
                                                scalar1=0.0)
                    nc.scalar.activation(neg[:], neg[:], AF.Exp)
                    pos = sb.tile([P, P], F32, tag="l1ps")
                    nc.vector.tensor_scalar_max(out=pos[:],
                                                in0=hps[:, blk, :],
                                                scalar1=0.0)
                    nc.vector.tensor_tensor(out=pos[:], in0=pos[:],
                                            in1=neg[:], op=OP.add)
                    nc.vector.tensor_scalar_add(out=x2t[:, blk, :],
                                                in0=pos[:], scalar1=-1.0)
                h2p = pp.tile([P, 264], F32, tag="gp")
                for c in range(2):
                    nc.tensor.matmul(out=h2p[:, 0:CE2U], lhsT=x2t[:, c, :],
                                     rhs=W2bs[:, c, :],
                                     start=(c == 0), stop=(c == 1))
                h2s = sb.tile([P, 68], BF16, tag="l1h2")
                nc.vector.tensor_copy(out=h2s[:, 0:64], in_=h2p[:, 0:64])
                nc.vector.tensor_copy(out=h2s[:, 64:68].bitcast(F32),
                                      in_=h2p[:, 64:66])
                nc.vector.tensor_tensor(out=rmax2[:], in0=rmax2[:],
                                        in1=h2p[:, 64:66], op=OP.max)
                nc.sync.dma_start(out=t2own[ds(w * P, P), 0:68],
                                  in_=h2s[:, 0:68])

            NGB = cfg.get("NGBUF", 2)
            g1bufs = [gbp.tile([P, TT, CE1], BF16, tag=f"g1_{i}", name=f"g1_{i}")
                      for i in range(NGB)]
            for g in g1bufs:
                nc.vector.memset(g[:, :, :], 0.0)

            def win1(w, k):
                sgb = sgp.tile([P, TT, CE1U], BF16, tag="sgb")
                wps, rec = gat_window(w, 1, cregs[k], g1bufs[k % NGB], sgb,
                                      qbase=k)
                if wps is not None:
                    l1_tail(w, wps, rec)

            if ABL != "nowin":
                for k in range(U):  # peeled (program warms the buffers)
                    win1(k, k)
                with tc.For_i(1, W // U) as w0:
                    for k in range(U):
                        win1(w0 * U + k, k)

            # ========== M2 bound (allreduce) + table2 allgather ============
            rr2 = sb.tile([P, 2], F32, tag="rr2")
            nc.gpsimd.partition_all_reduce(rr2[:], rmax2[:], P, ReduceOp.max)
            nc.sync.dma_start(out=m2loc[:, :], in_=rr2[0:1, :])
            nc.gpsimd.collective_compute(
                "AllReduce", OP.max, replica_groups=groups,
                ins=[m2loc[:, :]], outs=[m2sh[:, :]])
            m2t = sb.tile([P, 2], F32, tag="m2t")
            nc.sync.dma_start(out=m2t[:], in_=m2sh[:, :].to_broadcast([P, 2]))
            nc.vector.tensor_tensor(out=m2t[:, 0:1], in0=m2t[:, 0:1],
                                    in1=m2t[:, 1:2], op=OP.add)
            _leaky_max(nc, sb, m2t[:, 0:1], "m2lk")
            nc.vector.tensor_scalar_mul(out=mneg2[:], in0=m2t[:, 0:1],
                                        scalar1=-1.0)

            nc.gpsimd.collective_compute(
                "AllGather", OP.bypass, replica_groups=groups,
                ins=[t2own[:, :]], outs=[t2full[0:CORES * NCN, :]])

            # ========== layer-2 edge windows ===============================
            def l2_tail(w, wps, rec):
                onr = sb.tile([P, EMB], F32, tag="l2on")
                nc.vector.tensor_tensor(
                    out=onr[:], in0=wps[:, 0:EMB],
                    in1=rec[:].to_broadcast([P, EMB]), op=OP.mult)
                xot = sb.tile([P, EMB], F32, tag="l2xo")
                nc.sync.dma_start(out=xot[:], in_=xo_in[ds(w * P, P), :])
                nc.vector.tensor_tensor(out=xot[:], in0=xot[:], in1=onr[:],
                                        op=OP.add)
                nc.sync.dma_start(out=hown[ds(w * P, P), :], in_=xot[:])

            g2bufs = [gbp.tile([P, TT, CE2], BF16, tag=f"g2_{i}", name=f"g2_{i}")
                      for i in range(NGB)]
            for g in g2bufs:
                nc.vector.memset(g[:, :, :], 0.0)

            def win2(w, k):
                wps, rec = gat_window(w, 2, cregs[k], g2bufs[k % NGB], None,
                                      qbase=k)
                if wps is not None:
                    l2_tail(w, wps, rec)

            if ABL not in ("nowin", "now2"):
                for k in range(U):
                    win2(k, k)
                with tc.For_i(1, W // U) as w1:
                    for k in range(U):
                        win2(w1 * U + k, k)

            # ========== final embeddings allgather + dots ==================
            nc.gpsimd.collective_compute(
                "AllGather", OP.bypass, replica_groups=groups,
                ins=[hown[:, :]], outs=[hfull[:, :]])

            uix = sb.tile([P, BB // P], I32, tag="uix")
            nc.sync.dma_start(out=uix[:], in_=uidx_in[:, :])
            iix = sb.tile([P, BB // P], I32, tag="iix")
            nc.sync.dma_start(out=iix[:], in_=iidx_in[:, :])
            ubuf = gbp.tile([P, BB // P, EMB], F32, tag="ubuf", name="ubuf")
            ibuf = gbp.tile([P, BB // P, EMB], F32, tag="ibuf", name="ibuf")
            for j in range(BB // P):
                nc.gpsimd.indirect_dma_start(
                    out=ubuf[:, j, :], out_offset=None, in_=hfull[:, :],
                    in_offset=IndirectOffsetOnAxis(ap=uix[:, j:j + 1], axis=0))
                nc.gpsimd.indirect_dma_start(
                    out=ibuf[:, j, :], out_offset=None, in_=hfull[:, :],
                    in_offset=IndirectOffsetOnAxis(ap=iix[:, j:j + 1], axis=0))
            nc.vector.tensor_tensor(
                out=ubuf[:, :, :], in0=ubuf[:, :, :], in1=ibuf[:, :, :],
                op=OP.mult)
            dots = sb.tile([P, BB // P], F32, tag="dots")
            nc.vector.reduce_sum(dots[:], ubuf[:, :, :],
                                 axis=mybir.AxisListType.X)
            nc.sync.dma_start(out=out_t[:, :], in_=dots[:])

    nc.compile()
    return nc


def prepare_inputs(user_table, item_table, W1, a1, W2, a2, edge_index,
                   user_ids, item_ids, cfg):
    W, CORES = cfg["W"], cfg["CORES"]
    NSTAR, NCN, BB = cfg["NSTAR"], cfg["NC_NODES"], cfg["BB"]
    NU = user_table.shape[0]
    N = NU + item_table.shape[0]

    x = np.concatenate([np.asarray(user_table, np.float32),
                        np.asarray(item_table, np.float32)], axis=0)
    xpad = np.zeros((NSTAR, EMB), np.float32)
    xpad[:N] = x
    xT = np.ascontiguousarray(xpad.T)
    xr = xpad.astype(BF)

    W1 = np.asarray(W1, np.float32)
    a1 = np.asarray(a1, np.float32)
    W2 = np.asarray(W2, np.float32)
    a2 = np.asarray(a2, np.float32)
    A1l = np.stack([W1[:, h * EMB:(h + 1) * EMB] @ a1[h, :EMB]
                    for h in range(H1)], axis=1)
    A1r = np.stack([W1[:, h * EMB:(h + 1) * EMB] @ a1[h, EMB:]
                    for h in range(H1)], axis=1)
    W1e = np.concatenate([A1l, A1r], axis=1)          # [64, 8]
    # W1b[p, blk*64+f] = W1[p%64, (2*blk + p//64)*64 + f]  (head h at
    # partitions (h%2)*64 so the per-head matmul lhsT/rhs bases match)
    W1b = np.zeros((P, 2 * EMB), np.float32)
    for h in range(H1):
        blk, half = h // 2, (h % 2) * EMB
        W1b[half:half + EMB, blk * EMB:(blk + 1) * EMB] = \
            W1[:, h * EMB:(h + 1) * EMB]
    W1b = W1b.astype(BF)
    w2l = W2 @ a2[0, :EMB]
    w2r = W2 @ a2[0, EMB:]
    W2b = np.concatenate([W2, w2l[:, None], w2r[:, None]], axis=1).astype(BF)

    src = np.asarray(edge_index[0]).astype(np.int64)
    dst = np.asarray(edge_index[1]).astype(np.int64)
    NB = cfg["NB"]
    NWG = CORES * W
    key = (dst // P) * NB + (src // BANK)
    order = np.argsort(key, kind="stable")
    src_g, dst_g, key_g = src[order], dst[order], key[order]
    cnt = np.bincount(key_g, minlength=NWG * NB)
    Tb = cfg["Tb"]
    assert cnt.max() <= Tb * P, f"Tb={Tb} too small for {cnt.max()}"
    NIDXB = Tb * P
    gstart = np.concatenate([[0], np.cumsum(cnt)])[:-1]
    off = np.arange(len(src_g)) - gstart[key_g]
    slot = key_g * NIDXB + off
    bankidx = (src_g - (src_g // BANK) * BANK).astype(np.int16)

    def idx_layout(flat):
        A = flat.reshape(NWG, NB, Tb * 8, 16)
        A = np.transpose(A, (0, 1, 3, 2))                  # [wg, b, 16, cols]
        A = np.tile(A, (1, 1, 8, 1))                       # [wg, b, 128, cols]
        return np.ascontiguousarray(
            np.transpose(A, (0, 2, 1, 3)).reshape(NWG, P, NB * Tb * 8))

    if STATIC_COUNTS == "round128":
        # Dynamic counts rounded up to a multiple of 128: every SDMA engine
        # participates (sem reaches 16). Contract (bass_interp:3897): the
        # count register == #non-negative idxs and all idxs past it are -1.
        # Positions [cnt, cnt_r) gather row 0 (valid); tiles past cnt_r stay
        # stale-but-finite and their dstf=-1 zeroes their S columns.
        cnts = np.clip(cnt.reshape(NWG, NB), 128, NIDXB).astype(np.int32)
        idx_flatn = np.full(NWG * NB * NIDXB, -1, np.int16)
        idx_flatn[slot] = bankidx
        pos = np.tile(np.arange(NIDXB, dtype=np.int32), NWG * NB)
        inpad = (pos < cnts.reshape(-1).repeat(NIDXB)) & (idx_flatn < 0)
        idx_flatn[inpad] = 0
    else:
        idx_flatn = np.zeros(NWG * NB * NIDXB, np.int16)
        idx_flatn[slot] = bankidx
        cnts = np.full((NWG, NB), NIDXB, np.int32)
    idx_dma_n = idx_layout(idx_flatn)
    dst_flat = np.full(NWG * NB * NIDXB, -1.0, np.float32)
    dst_flat[slot] = (dst_g % P).astype(np.float32)
    dst_dma = np.ascontiguousarray(
        dst_flat.reshape(NWG, NB * Tb, P).transpose(0, 2, 1))

    iota_np = np.tile(np.arange(P, dtype=np.float32), (P, 1))
    ident_np = np.eye(P, dtype=np.float32)

    uids = np.asarray(user_ids).astype(np.int64)
    iids = np.asarray(item_ids).astype(np.int64) + NU

    in_maps = []
    for k in range(CORES):
        in_maps.append(dict(
            xT=xT,
            xr=xr,
            xTo=np.ascontiguousarray(xT[:, k * NCN:(k + 1) * NCN]),
            xo=np.ascontiguousarray(xpad[k * NCN:(k + 1) * NCN]),
            W1e=W1e, W1b=W1b, W2b=W2b, iota=iota_np, ident=ident_np,
            dstw=idx_dst(idx_dma_n, dst_dma, k, W)[1],
            idxw=idx_dst(idx_dma_n, dst_dma, k, W)[0],
            cnts=np.ascontiguousarray(cnts[k * W:(k + 1) * W]),
            cnts2=np.ascontiguousarray(cnts[k * W:(k + 1) * W]),
            uidx=uids[k * cfg["BB"]:(k + 1) * cfg["BB"]].astype(
                np.int32).reshape(P, BB // P),
            iidx=iids[k * cfg["BB"]:(k + 1) * cfg["BB"]].astype(
                np.int32).reshape(P, BB // P),
        ))
    return in_maps


def idx_dst(idx_dma, dst_dma, k, W):
    i = np.ascontiguousarray(
        idx_dma[k * W:(k + 1) * W].reshape(W * P, -1))
    d = np.ascontiguousarray(
        dst_dma[k * W:(k + 1) * W].reshape(W * P, -1))
    return i, d


DEFAULT_CFG = dict(
    CORES=8, W=148, NC_NODES=148 * P, NSTAR=8 * 148 * P, NTAB=NBANKS * BANK,
    NB=5, Tb=4, BB=2048, UN_D1=16, UN_D1B=4, UN_WIN=4, UN_D2=4, NGBUF=4,
    NQ=4,
)

_PROGRAM_CACHE = {}


def _get_program(cfg_key, cfg):
    if cfg_key not in _PROGRAM_CACHE:
        _PROGRAM_CACHE[cfg_key] = build_program(cfg)
    return _PROGRAM_CACHE[cfg_key]


def run(inputs, cfg=None, trace=False):
    cfg = dict(DEFAULT_CFG if cfg is None else cfg)
    # size Tb from the data (static program structure depends on it)
    src = np.asarray(inputs["edge_index"][0]).astype(np.int64)
    dst = np.asarray(inputs["edge_index"][1]).astype(np.int64)
    key = (dst // P) * cfg["NB"] + (src // BANK)
    cnt = np.bincount(key, minlength=cfg["CORES"] * cfg["W"] * cfg["NB"])
    cfg["Tb"] = max(int(np.ceil(cnt.max() / P)), 1)
    in_maps = prepare_inputs(cfg=cfg, **inputs)
    nc = _get_program(tuple(sorted(cfg.items())), cfg)
    res = run_bass_kernel_spmd(nc, in_maps,
                               core_ids=list(range(cfg["CORES"])),
                               trace=trace)
    outs = [res.results[k]["out"].reshape(-1) for k in range(cfg["CORES"])]
    return np.concatenate(outs).astype(np.float32), res


def kernel(user_table, item_table, W1, a1, W2, a2, edge_index, user_ids,
           item_ids):
    out, _ = run(dict(user_table=user_table, item_table=item_table, W1=W1,
                      a1=a1, W2=W2, a2=a2, edge_index=edge_index,
                      user_ids=user_ids, item_ids=item_ids))
    return out

